# revision 15
# baseline (speedup 1.0000x reference)
"""Trainium2 Bass kernel for the MAMGCN encoder block.

Strategy: data-parallel over batch B=16 across 8 NeuronCores (2 batches/core).
Host-side prep (untimed): shard x, repack small weights, pre-transpose x to
(t*64+f, n) layout, cast matmul operands to bf16. Device does everything else:
spatial attention (two fused weight matmuls -> product -> tanh-sigmoid ->
Vs@P -> exp -> column softmax), Chebyshev graph conv with Theta folded in
(Y = X @ Theta2 block-diag), all matmuls in bf16 with fp32 PSUM accumulation.
"""
import numpy as np
import ml_dtypes

B, N, F, T, K, FO = 16, 1024, 64, 24, 3, 64
NCORES = 8
BPC = B // NCORES          # batches per core
NCH = N // 128             # 8 partition chunks of N
NJ = (T * F) // 128        # 12 chunks of the tf dim
THALF = T // 2             # 12
bf16 = ml_dtypes.bfloat16

_CACHE = {}


def _build_nc():
    import concourse.bacc as bacc
    import concourse.bass as bass
    import concourse.tile as tile
    import concourse.mybir as mybir

    fp32 = mybir.dt.float32
    bf = mybir.dt.bfloat16
    AF = mybir.ActivationFunctionType

    nc = bacc.Bacc(
        "TRN2", target_bir_lowering=False, debug=False,
        enable_asserts=True, num_devices=NCORES,
    )

    # ---- DRAM I/O ----
    x_d = nc.dram_tensor("x_tf", [BPC, NJ, 128, N], bf, kind="ExternalInput")
    bs_d = nc.dram_tensor("bs_t", [NCH, 128, N], bf, kind="ExternalInput")
    vs_d = nc.dram_tensor("vs_t", [NCH, 128, N], bf, kind="ExternalInput")
    cheb_d = nc.dram_tensor("cheb_t", [K, NCH, 128, N], bf, kind="ExternalInput")
    wcat_d = nc.dram_tensor("wcat", [NJ, 128, 2 * T], bf, kind="ExternalInput")
    th2_d = nc.dram_tensor("th2", [128, 2 * K * FO], bf, kind="ExternalInput")
    hrow_d = nc.dram_tensor("hrow", [NCH, 128, 1], fp32, kind="ExternalInput")
    # out[b, half, mchunk, p, o, tl]
    out_d = nc.dram_tensor("out", [BPC, 2, NCH, 128, FO, THALF], fp32,
                           kind="ExternalOutput")

    with tile.TileContext(nc) as tc:
        with (
            tc.tile_pool(name="const", bufs=1) as cpool,
            tc.tile_pool(name="work", bufs=2) as wpool,
            tc.tile_pool(name="big", bufs=1) as bpool,
            tc.tile_pool(name="psA", bufs=2, space="PSUM") as psA,
            tc.tile_pool(name="psB", bufs=2, space="PSUM") as psB,
        ):
            # ---- constants ----
            vsT_sb = cpool.tile([128, NCH, N], bf, tag="vsT")
            bs_sb = cpool.tile([128, NCH, N], bf, tag="bs")
            wcat_sb = cpool.tile([128, NJ, 2 * T], bf, tag="wcat")
            th2_sb = cpool.tile([128, 2 * K * FO], bf, tag="th2")
            hrow_sb = cpool.tile([128, NCH], fp32, tag="hrow")
            ones_sb = cpool.tile([128, 1], bf, tag="ones")
            one1_sb = cpool.tile([1, 1], fp32, tag="one1")
            for c in range(NCH):
                nc.sync.dma_start(vsT_sb[:, c, :], vs_d[c])
                nc.sync.dma_start(bs_sb[:, c, :], bs_d[c])
                nc.sync.dma_start(hrow_sb[:, c:c + 1], hrow_d[c])
            for j in range(NJ):
                nc.sync.dma_start(wcat_sb[:, j, :], wcat_d[j])
            nc.sync.dma_start(th2_sb[:], th2_d[:])
            nc.gpsimd.memset(ones_sb[:], 1.0)
            nc.gpsimd.memset(one1_sb[:], 1.0)

            # ---- per-batch state (single-buffered big tiles) ----
            for b in range(BPC):
                x_sb = bpool.tile([128, NJ, N], bf, tag="x")
                e_sb = bpool.tile([128, NCH, N], bf, tag="e")
                p_sb = bpool.tile([128, NCH, N], bf, tag="p")
                a_sb = bpool.tile([128, K, NCH, N], bf, tag="a")
                y_sb = bpool.tile([128, NCH, K, THALF, FO], bf, tag="y")
                rT_sb = bpool.tile([128, NCH], fp32, tag="rT")

                for j0 in range(0, NJ, 4):
                    nc.sync.dma_start(
                        x_sb[:, j0:j0 + 4, :],
                        x_d[b, j0:j0 + 4].rearrange("j p n -> p j n"))

                # ---- attention pre-reductions (one pass over x) ----
                att_c = wpool.tile([2 * T, N], bf, tag="attc")
                att_r = wpool.tile([T, N], bf, tag="attr")
                pa = psA.tile([2 * T, N], fp32, tag="big")
                for j in range(NJ):
                    for s in range(2):
                        nc.tensor.matmul(
                            pa[:, s * 512:(s + 1) * 512],
                            wcat_sb[:, j, :],
                            x_sb[:, j, s * 512:(s + 1) * 512],
                            start=(j == 0), stop=(j == NJ - 1),
                        )
                nc.scalar.copy(att_c[:], pa[:])
                # shift rows 24..47 down to partitions 0..23 for the product
                nc.sync.dma_start(att_r[:], att_c[T:2 * T, :])
                att_l = att_c

                # ---- product + bs -> tanh(0.5*) -> P ----
                for cn in range(NCH):
                    pp = psA.tile([128, N], fp32, tag="big")
                    for s in range(2):
                        nc.tensor.matmul(
                            pp[:, s * 512:(s + 1) * 512],
                            att_l[0:T, cn * 128:(cn + 1) * 128],
                            att_r[:, s * 512:(s + 1) * 512],
                            start=True, stop=True,
                        )
                    tmp = wpool.tile([128, N], bf, tag="tmp")
                    nc.vector.tensor_add(tmp[:], pp[:], bs_sb[:, cn, :])
                    nc.scalar.activation(p_sb[:, cn, :], tmp[:], AF.Tanh, scale=0.5)

                # ---- S_pre = Vs @ P (per i-chunk), exp -> E ----
                for ic in range(NCH):
                    ps = psA.tile([128, N], fp32, tag="big")
                    for kc in range(NCH):
                        for s in range(2):
                            nc.tensor.matmul(
                                ps[:, s * 512:(s + 1) * 512],
                                vsT_sb[:, kc, ic * 128:(ic + 1) * 128],
                                p_sb[:, kc, s * 512:(s + 1) * 512],
                                start=(kc == 0), stop=(kc == NCH - 1),
                            )
                    nc.scalar.activation(
                        e_sb[:, ic, :], ps[:], AF.Exp,
                        scale=0.5, bias=hrow_sb[:, ic:ic + 1],
                    )

                # ---- column sums of E -> recip -> rT (128, 8) ----
                pc = psA.tile([1, N], fp32, tag="big")
                for ic in range(NCH):
                    for s in range(2):
                        nc.tensor.matmul(
                            pc[:, s * 512:(s + 1) * 512],
                            ones_sb[:],
                            e_sb[:, ic, s * 512:(s + 1) * 512],
                            start=(ic == 0), stop=(ic == NCH - 1),
                        )
                csum_sb = wpool.tile([1, N], fp32, tag="csum")
                nc.scalar.copy(csum_sb[:], pc[:])
                prt = psA.tile([128, NCH], fp32, tag="big")
                for c in range(NCH):
                    nc.tensor.matmul(
                        prt[:, c:c + 1],
                        csum_sb[:, c * 128:(c + 1) * 128],
                        one1_sb[:],
                        start=True, stop=True,
                    )
                nc.vector.reciprocal(rT_sb[:], prt[:])

                # ---- A = cheb * E (bf16), cheb streamed from HBM ----
                for k in range(K):
                    for cn in range(NCH):
                        ch = wpool.tile([128, N], bf, tag="cheb", bufs=3)
                        nc.sync.dma_start(ch[:], cheb_d[k, cn])
                        nc.vector.tensor_mul(a_sb[:, k, cn, :], ch[:], e_sb[:, cn, :])

                # ---- two t-halves: Y build + graph conv ----
                for h in range(2):
                    for j in range(6 * h, 6 * h + 6):
                        tl0 = 2 * (j - 6 * h)
                        for cn in range(NCH):
                            py = psB.tile([128, 2, K, FO], fp32, tag="out")
                            nc.tensor.matmul(
                                py[:, :, :, :],
                                x_sb[:, j, cn * 128:(cn + 1) * 128],
                                th2_sb[:],
                                start=True, stop=True,
                            )
                            nc.vector.tensor_copy(
                                y_sb[:, cn, :, tl0, :], py[:, 0, :, :])
                            nc.scalar.copy(
                                y_sb[:, cn, :, tl0 + 1, :], py[:, 1, :, :])
                    for mc in range(NCH):
                        po = psB.tile([128, THALF, FO], fp32, tag="out")
                        nmm = 0
                        for k in range(K):
                            for cn in range(NCH):
                                first = nmm == 0
                                last = nmm == K * NCH - 1
                                nc.tensor.matmul(
                                    po[:, 0:8, :],
                                    a_sb[:, k, cn, mc * 128:(mc + 1) * 128],
                                    y_sb[:, cn, k, 0:8, :],
                                    start=first, stop=last,
                                )
                                nc.tensor.matmul(
                                    po[:, 8:THALF, :],
                                    a_sb[:, k, cn, mc * 128:(mc + 1) * 128],
                                    y_sb[:, cn, k, 8:THALF, :],
                                    start=first, stop=last,
                                )
                                nmm += 1
                        st = wpool.tile([128, FO, THALF], fp32, tag="stage")
                        nc.scalar.activation(
                            st[:],
                            po[:, :, :].rearrange("p t o -> p o t"),
                            AF.Relu,
                            scale=rT_sb[:, mc:mc + 1],
                        )
                        nc.sync.dma_start(out_d[b, h, mc], st[:])

    nc.compile()
    return nc


def _host_prep(x, W1, W2, W3, bs, Vs, cheb, Theta):
    x = np.asarray(x, np.float32)
    W1 = np.asarray(W1, np.float32)
    W2 = np.asarray(W2, np.float32)
    W3 = np.asarray(W3, np.float32)
    bs = np.asarray(bs, np.float32)
    Vs = np.asarray(Vs, np.float32)
    cheb = np.asarray(cheb, np.float32)
    Theta = np.asarray(Theta, np.float32)

    x_tf = np.ascontiguousarray(x.transpose(0, 3, 2, 1)).reshape(B, NJ, 128, N)
    x_tf = x_tf.astype(bf16)
    bs_t = bs[0].reshape(NCH, 128, N).astype(bf16)
    vs_t = np.ascontiguousarray(Vs.T).reshape(NCH, 128, N).astype(bf16)
    cheb_t = cheb.reshape(K, NCH, 128, N).astype(bf16)
    t_idx = np.arange(T * F) // F
    f_idx = np.arange(T * F) % F
    wl_flat = W1[t_idx][:, None] * W2[f_idx, :]
    wr_flat = np.zeros((T * F, T), np.float32)
    wr_flat[np.arange(T * F), t_idx] = W3[f_idx]
    wcat = np.concatenate([wl_flat, wr_flat], axis=1)
    wcat = wcat.reshape(NJ, 128, 2 * T).astype(bf16)
    th2 = np.zeros((128, 2 * K * FO), np.float32)
    for par in range(2):
        for k in range(K):
            th2[par * F:(par + 1) * F,
                par * K * FO + k * FO:(par * K + k + 1) * FO] = Theta[k]
    th2 = th2.astype(bf16)
    hrow = (0.5 * Vs.sum(axis=1)).astype(np.float32).reshape(NCH, 128, 1)
    return x_tf, bs_t, vs_t, cheb_t, wcat, th2, hrow


def kernel(x, W1, W2, W3, bs, Vs, cheb, Theta, _return_results=False,
           _trace=False):
    from concourse.bass_utils import run_bass_kernel_spmd

    x_tf, bs_t, vs_t, cheb_t, wcat, th2, hrow = _host_prep(
        x, W1, W2, W3, bs, Vs, cheb, Theta)

    if "nc" not in _CACHE:
        _CACHE["nc"] = _build_nc()
    nc = _CACHE["nc"]

    shared = dict(bs_t=bs_t, vs_t=vs_t, cheb_t=cheb_t, wcat=wcat,
                  th2=th2, hrow=hrow)
    in_maps = []
    for c in range(NCORES):
        m = dict(shared)
        m["x_tf"] = np.ascontiguousarray(x_tf[c * BPC:(c + 1) * BPC])
        in_maps.append(m)

    _CACHE["in_maps"] = in_maps
    kw = {"trace": True} if _trace else {}
    res = run_bass_kernel_spmd(nc, in_maps, list(range(NCORES)), **kw)
    outs = []
    for c in range(NCORES):
        o = res.results[c]["out"]  # (BPC, 2, NCH, 128, FO, THALF)
        o = o.transpose(0, 2, 3, 4, 1, 5).reshape(BPC, N, FO, T)
        outs.append(o)
    full = np.concatenate(outs, axis=0).astype(np.float32)
    if _return_results:
        return full, res
    return full
